# revision 9
# baseline (speedup 1.0000x reference)
"""Trainium2 Bass kernel for nn_BaseAttention (B=4, N=M=4096, C=256, R=512).

  q = x @ Wq.T;  k = ref @ Wk.T;  v = ref @ Wv.T
  out = softmax(q @ k.T / sqrt(C)) @ v @ Wo.T

Sharding: 8 cores; core i handles batch i//2, query rows (i%2)*2048..+2048.
K/V projection work is duplicated across the 2 cores of a batch (cheap).

v4: fp8(e4m3) DoubleRow matmuls for the two dominant phases (scores and
P@V run 2 fp8 MACs/cell/cycle). Precision is held by an expm1-style
decomposition: P' = 8*(exp(s)-1) is quantized to fp8 (error lands on the
small fluctuation term, not softmax's O(1) mean), P'@V'' accumulates in
fp8, and the exact rank-1 correction colsum(V_bf16) (ones^T @ VAb
matmuls) restores both the softmax mean term and the fp8-V quantization
loss. The k-projection stays bf16 (fp8 there pushes rel-err over the
gate); only the kT eviction quantizes to fp8. Scale factors (x16 on Wk,
x8 on Wv) keep fp8 operands in e4m3's normal range and are folded into
host weight prep, the exp scale, and the final output scale.

Schedule (all PSUM through one 2-buffer ring of [128,2048] tiles):
  Phase 1 (8 stripes, skew-2 software pipeline): projections for stripe
  s+2 + scores/exp/sub for q-blocks 0-1 of stripe s. Colsum rides the
  last iteration's exp gaps.
  Phase 2: scores for q-blocks 2-3 pipelined against P@V of q-blocks
  0-2; P@V groups borrow ring slots ([0:258] slice). P@V of q-block 3
  drains as the tail.
Pointwise split: exp on ScalarE, (x-1)*8->fp8 subs on DVE + 1-in-4 on
GpSimd, PSUM evictions on DVE, VAb->VA fp8 cast + final scaled muls on
GpSimd DMAs/ALUs.
"""

import sys

sys.path.insert(0, "/opt/trn_rl_repo")

import ml_dtypes
import numpy as np

import concourse.bass as bass
import concourse.mybir as mybir
import concourse.tile as tile
from concourse import bacc
from concourse.bass_utils import run_bass_kernel_spmd

B = 4
N = 4096
M = 4096
C = 256  # INPUT_CH
R = 512  # REF_CH
SCALE = C ** (-0.5)
KSC = 16.0  # Wk host prescale
VSC = 8.0  # Wv host prescale
LAM = SCALE / KSC  # exp() scale on the raw score psum
NQ = 2048  # query rows per core

F32 = mybir.dt.float32
BF16 = mybir.dt.bfloat16
F8 = mybir.dt.float8e4
NP_BF16 = ml_dtypes.bfloat16
NP_F8 = ml_dtypes.float8_e4m3
DR = mybir.MatmulPerfMode.DoubleRow
Exp = mybir.ActivationFunctionType.Exp
Copy = mybir.ActivationFunctionType.Copy
ALU = mybir.AluOpType

QB = 512  # query block (free dim of score matmuls)
N_QB = NQ // QB  # 4
N_MC = M // 128  # 32 key chunks
N_CC = C // 128  # 2 chunks of the model dim
N_RC = R // 128  # 4 chunks of the ref dim
STRIPE = 512  # ref rows per processing stripe
N_STRIPES = M // STRIPE  # 8
VROW = 272  # VA chunk stride (C+2 used, padded to a 16B multiple)

_cached = None


def _build():
    nc = bacc.Bacc("TRN2", target_bir_lowering=False, debug=False)

    xT8_d = nc.dram_tensor("xT8", [C, NQ], F8, kind="ExternalInput")
    refT_d = nc.dram_tensor("refT", [R, M], BF16, kind="ExternalInput")
    wq_d = nc.dram_tensor("wq", [C, C], BF16, kind="ExternalInput")
    wk_d = nc.dram_tensor("wk16", [C, R], BF16, kind="ExternalInput")
    wv_d = nc.dram_tensor("wv8", [C, R], BF16, kind="ExternalInput")
    woT_d = nc.dram_tensor("woT", [C, C], BF16, kind="ExternalInput")
    out_d = nc.dram_tensor("out", [NQ, C], F32, kind="ExternalOutput")

    scratch_d = nc.dram_tensor("scratch", [128, 2], F32)

    with tile.TileContext(nc) as tc:
        with tc.tile_pool(name="const", bufs=1) as pc:
            # Persistent tiles
            kT8 = pc.tile([128, N_CC, M], F8)  # 16*k''^T  [c, m]
            VA = pc.tile([128, N_MC, VROW], F8)  # [8V' | 1 | 1 | pad]
            VAb = pc.tile([128, N_MC, C + 2], BF16)  # bf16 copy for colsum
            xT8 = pc.tile([128, N_CC, NQ], F8)
            gT = pc.tile([128, N_RC, C], BF16)  # 16*G^T = (16Wk)^T @ Wq
            wvoT = pc.tile([128, N_RC, C], BF16)  # (8 Wvo)^T
            caug_sb = pc.tile([1, C + 2], F32)
            caug_b = pc.tile([128, C + 2], F32)
            ones1 = pc.tile([128, 1], BF16)

            _psS_cm = tc.tile_pool(name="psS", bufs=2, space="PSUM")
            _pst_cm = tc.tile_pool(name="stage", bufs=2)
            _pstg_cm = tc.tile_pool(name="expstage", bufs=3)
            _ppt_cm = tc.tile_pool(name="ptpool", bufs=3)
            _pout_cm = tc.tile_pool(name="attn_out", bufs=4)
            psS = _psS_cm.__enter__()
            pst = _pst_cm.__enter__()
            pstg = _pstg_cm.__enter__()
            ppt = _ppt_cm.__enter__()
            pout = _pout_cm.__enter__()

            nc.gpsimd.memset(VA[:, :, C : C + 2], 1.0)
            nc.gpsimd.memset(VAb[:, :, C : C + 2], 1.0)
            nc.gpsimd.memset(ones1[:], 1.0)

            # --- PE warm-up: fills the otherwise-idle input-DMA wait window
            # with matmul activity so the HAM clock gate is already at K=8/8
            # (2.4 GHz) when the first projection matmul issues.
            wu = pst.tile([128, QB], BF16, tag="wu", bufs=1)
            nc.vector.memset(wu[:], 0.0)
            ps_wu = psS.tile([128, 4 * QB], F32, tag="sps")
            for _ in range(13):
                nc.tensor.matmul(
                    ps_wu[:, 0:QB], wu[:, 0:128], wu[:], start=True, stop=True
                )
            wu_out = pst.tile([128, 2], F32, tag="wu_out", bufs=1)
            nc.vector.tensor_copy(wu_out[:], ps_wu[:, 0:2])
            nc.sync.dma_start(scratch_d[:], wu_out[:])

            # ---------------- weight loads (pre-transposed on host) -------
            wq = pst.tile([128, N_CC, C], BF16, tag="wq", bufs=1)
            nc.scalar.dma_start(wq[:], wq_d[:].rearrange("(a p) o -> p a o", p=128))
            wk = pst.tile([128, N_CC, R], BF16, tag="wk", bufs=1)
            nc.scalar.dma_start(wk[:], wk_d[:].rearrange("(a p) r -> p a r", p=128))
            wv = pst.tile([128, N_CC, R], BF16, tag="wv", bufs=1)
            nc.scalar.dma_start(wv[:], wv_d[:].rearrange("(a p) r -> p a r", p=128))
            woT = pst.tile([128, N_CC, C], BF16, tag="woT", bufs=1)
            nc.scalar.dma_start(woT[:], woT_d[:].rearrange("(a p) o -> p a o", p=128))

            # xT8 on the ACT HWDGE ring so it doesn't delay the refT stripes
            nc.scalar.dma_start(xT8[:], xT8_d[:].rearrange("(j p) n -> p j n", p=128))

            # weight folds: gT[r,c] = sum_co 16Wk[co,r] Wq[co,c];
            # WvoT[r,c'] = sum_c 8Wv[c,r] Wo[c',c]. One psum tile, 8 groups.
            ps_f = psS.tile([128, 4 * QB], F32, tag="sps", name="ps")
            for rj in range(N_RC):
                for a in range(N_CC):
                    nc.tensor.matmul(
                        ps_f[:, rj * C : (rj + 1) * C],
                        wk[:, a, rj * 128 : (rj + 1) * 128],
                        wq[:, a, :],
                        start=(a == 0),
                        stop=(a == N_CC - 1),
                    )
            for rj in range(N_RC):
                for a in range(N_CC):
                    nc.tensor.matmul(
                        ps_f[:, 4 * C + rj * C : 4 * C + (rj + 1) * C],
                        wv[:, a, rj * 128 : (rj + 1) * 128],
                        woT[:, a, :],
                        start=(a == 0),
                        stop=(a == N_CC - 1),
                    )
            nc.scalar.activation(
                gT[:], ps_f[:, 0 : 4 * C].rearrange("p (a c) -> p a c", a=4), Copy
            )
            nc.scalar.activation(
                wvoT[:],
                ps_f[:, 4 * C : 8 * C].rearrange("p (a c) -> p a c", a=4),
                Copy,
            )

            # PT tiles: 8*(P-1)^T per q-block, ring of 3 (qb uses slot qb%3;
            # qb3's alloc waits until P@V of qb0 has drained -- by then done)
            PT = {}

            refT_tiles = {}

            def dma_refT(s):
                if s >= N_STRIPES:
                    return
                t = pst.tile([128, N_RC, STRIPE], BF16, tag="refT", bufs=4)
                m0 = s * STRIPE
                nc.sync.dma_start(
                    t[:],
                    refT_d[:, m0 : m0 + STRIPE].rearrange("(j p) m -> p j m", p=128),
                )
                refT_tiles[s] = t

            def proj(s):
                # kT and V' projections for stripe s + their evictions.
                # One [128,2048] psum tile: [kT a0 | kT a1 | V' mi0..3]
                if s >= N_STRIPES:
                    return
                refT = refT_tiles.pop(s)
                m0 = s * STRIPE
                ps = psS.tile([128, 4 * QB], F32, tag="sps", name="ps")
                for a in range(N_CC):
                    for j in range(N_RC):
                        nc.tensor.matmul(
                            ps[:, a * 512 : (a + 1) * 512],
                            gT[:, j, a * 128 : (a + 1) * 128],
                            refT[:, j, :],
                            start=(j == 0),
                            stop=(j == N_RC - 1),
                        )
                for mi in range(4):
                    for j in range(N_RC):
                        nc.tensor.matmul(
                            ps[:, 1024 + mi * C : 1024 + (mi + 1) * C],
                            refT[:, j, mi * 128 : (mi + 1) * 128],
                            wvoT[:, j, :],
                            start=(j == 0),
                            stop=(j == N_RC - 1),
                        )
                nc.vector.tensor_copy(
                    kT8[:, 0:2, m0 : m0 + STRIPE],
                    ps[:, 0:1024].rearrange("p (a m) -> p a m", a=2),
                )
                mc0 = 4 * s
                nc.vector.tensor_copy(
                    VAb[:, mc0 : mc0 + 4, 0:C],
                    ps[:, 1024:2048].rearrange("p (a c) -> p a c", a=4),
                )
                nc.gpsimd.dma_start(
                    VA[:, mc0 : mc0 + 4, 0:C], VAb[:, mc0 : mc0 + 4, 0:C]
                )

            def scores_tile(qb, g, on_gp=False):
                # scores for q-block qb vs key chunks 4g..4g+3: 4 DR matmuls
                # into one [128,2048] psum tile, exp, (x-1)*8 -> fp8 PT[qb]
                ps = psS.tile([128, 4 * QB], F32, tag="sps", name="ps")
                for mcl in range(4):
                    mc = 4 * g + mcl
                    nc.tensor.matmul(
                        ps[:, mcl * QB : (mcl + 1) * QB],
                        kT8[:, 0:2, mc * 128 : (mc + 1) * 128],
                        xT8[:, 0:2, qb * QB : (qb + 1) * QB],
                        start=True,
                        stop=True,
                        perf_mode=DR,
                    )
                stg = pstg.tile([128, 4 * QB], F32, tag="stg", bufs=3)
                nc.scalar.activation(stg[:], ps[:], Exp, scale=float(LAM))
                eng = nc.gpsimd if on_gp else nc.vector
                eng.tensor_scalar(
                    PT[qb][:, 4 * g : 4 * g + 4, :],
                    stg[:].rearrange("p (a q) -> p a q", a=4),
                    -1.0,
                    8.0,
                    op0=ALU.add,
                    op1=ALU.mult,
                )

            def pv_group(qb, qs):
                # P@V for 128 query rows: 16 fp8-DR matmuls in a ring slot
                ps = psS.tile([128, 4 * QB], F32, tag="sps", name="ps")
                for i in range(N_MC // 2):
                    nc.tensor.matmul(
                        ps[:, 0 : C + 2],
                        PT[qb][:, 2 * i : 2 * i + 2, qs * 128 : (qs + 1) * 128],
                        VA[:, 2 * i : 2 * i + 2, 0 : C + 2],
                        start=(i == 0),
                        stop=(i == N_MC // 2 - 1),
                        perf_mode=DR,
                    )
                numf = pout.tile([128, C + 2], F32, tag="numf", name="numf")
                nc.vector.scalar_tensor_tensor(
                    numf[:], ps[:, 0 : C + 2], 1.0, caug_b[:], op0=ALU.mult, op1=ALU.add
                )
                recip = pout.tile([128, 1], F32, tag="recip", name="recip")
                nc.vector.reciprocal(recip[:], numf[:, C : C + 1])
                o_sb = pout.tile([128, C], F32, tag="osb", name="o_sb")
                nc.gpsimd.tensor_scalar(
                    o_sb[:],
                    numf[:, 0:C],
                    recip[:],
                    1.0 / VSC,
                    op0=ALU.mult,
                    op1=ALU.mult,
                )
                r0 = qb * QB + qs * 128
                nc.sync.dma_start(out_d[r0 : r0 + 128, :], o_sb[:])

            # ---------------- phase 1: stripes, q-blocks 0-1 --------------
            PT[0] = ppt.tile([128, N_MC, QB], F8, tag="PT", name="PT0")
            PT[1] = ppt.tile([128, N_MC, QB], F8, tag="PT", name="PT1")
            dma_refT(0)
            dma_refT(1)
            dma_refT(2)
            proj(0)
            proj(1)
            for s in range(N_STRIPES):
                dma_refT(s + 3)
                scores_tile(0, s)
                scores_tile(1, s, on_gp=True)
                proj(s + 2)
                if s == N_STRIPES - 1:
                    # colsum(V'') via ones^T @ VAb: caug = 8 * colsum
                    csps = psS.tile([128, 4 * QB], F32, tag="sps", name="cs")
                    for mc in range(N_MC):
                        nc.tensor.matmul(
                            csps[0:1, 0 : C + 2],
                            ones1[:],
                            VAb[:, mc, :],
                            start=(mc == 0),
                            stop=(mc == N_MC - 1),
                        )
                    nc.scalar.activation(
                        caug_sb[0:1, :], csps[0:1, 0 : C + 2], Copy, scale=float(VSC)
                    )
                    nc.gpsimd.partition_broadcast(caug_b[:], caug_sb[:])

            # ------- phase 2: scores qb2/qb3 pipelined with P@V 0-2 -------
            PT[2] = ppt.tile([128, N_MC, QB], F8, tag="PT", name="PT2")
            for g in range(N_STRIPES):
                scores_tile(2, g, on_gp=(g % 4 == 3))
                pv_group(g // 4, g % 4)
            PT[3] = ppt.tile([128, N_MC, QB], F8, tag="PT", name="PT3")
            for g in range(N_STRIPES):
                scores_tile(3, g, on_gp=(g % 4 == 3))
                if g >= 4:
                    pv_group(2, g - 4)
            for qs in range(4):
                pv_group(3, qs)

            _pout_cm.__exit__(None, None, None)
            _ppt_cm.__exit__(None, None, None)
            _pstg_cm.__exit__(None, None, None)
            _pst_cm.__exit__(None, None, None)
            _psS_cm.__exit__(None, None, None)

    nc.compile()
    return nc


def _get_nc():
    global _cached
    if _cached is None:
        _cached = _build()
    return _cached


def kernel(x, ref, Wq, Wk, Wv, Wo, _trace=False, _trace_kwargs=None):
    nc = _get_nc()
    x = np.asarray(x, dtype=np.float32)
    ref = np.asarray(ref, dtype=np.float32)
    # host-side layout marshalling (transpose + cast + constant prescales on
    # the tiny weight tensors; no model FLOPs)
    wq_h = np.ascontiguousarray(np.asarray(Wq, np.float32).astype(NP_BF16))
    wk_h = np.ascontiguousarray((np.asarray(Wk, np.float32) * KSC).astype(NP_BF16))
    wv_h = np.ascontiguousarray((np.asarray(Wv, np.float32) * VSC).astype(NP_BF16))
    woT_h = np.ascontiguousarray(np.asarray(Wo, np.float32).T.astype(NP_BF16))
    refT_h = [np.ascontiguousarray(ref[b].T.astype(NP_BF16)) for b in range(B)]
    in_maps = []
    for core in range(8):
        b, h = divmod(core, 2)
        xT8_h = np.ascontiguousarray(x[b, h * NQ : (h + 1) * NQ, :].T.astype(NP_F8))
        in_maps.append(
            {
                "xT8": xT8_h,
                "refT": refT_h[b],
                "wq": wq_h,
                "wk16": wk_h,
                "wv8": wv_h,
                "woT": woT_h,
            }
        )
    res = run_bass_kernel_spmd(
        nc, in_maps, list(range(8)), trace=_trace, **(_trace_kwargs or {})
    )
    kernel.last_result = res
    out = np.empty((B, N, C), dtype=np.float32)
    for core in range(8):
        b, h = divmod(core, 2)
        out[b, h * NQ : (h + 1) * NQ, :] = res.results[core]["out"]
    return out


# revision 10
# speedup vs baseline: 1.0043x; 1.0043x over previous
"""Trainium2 Bass kernel for nn_BaseAttention (B=4, N=M=4096, C=256, R=512).

  q = x @ Wq.T;  k = ref @ Wk.T;  v = ref @ Wv.T
  out = softmax(q @ k.T / sqrt(C)) @ v @ Wo.T

Sharding: 8 cores; core i handles batch i//2, query rows (i%2)*2048..+2048.
K/V projection work is duplicated across the 2 cores of a batch (cheap).

v5: fp8(e4m3) DoubleRow matmuls for the two dominant phases (scores and
P@V run 2 fp8 MACs/cell/cycle). Precision is held by an expm1-style
decomposition: P' = 8*(exp(s)-1) is quantized to fp8 (error lands on the
small fluctuation term, not softmax's O(1) mean), P'@V'' accumulates in
fp8, and the exact rank-1 correction colsum(V_bf16) (ones^T @ VAb
matmuls) restores both the softmax mean term and the fp8-V quantization
loss. The k-projection stays bf16 (fp8 there pushes rel-err over the
gate); only the kT eviction quantizes to fp8. Scale factors (x16 on Wk,
x8 on Wv) keep fp8 operands in e4m3's normal range and are folded into
host weight prep, the exp scale, and the final output scale.

Schedule: PSUM = 3-deep ring of [128,1024] score tiles (6 banks) + a
2-deep [128,512] aux ring (2 banks) for projections / P@V / colsum, so
PE never blocks on an exp draining a ring slot. Phase 1 (8 stripes,
skew-2): scores for q-blocks 0-1 finely interleaved with stripe s+2's
projection pieces. Phase 2: scores for q-blocks 2-3 interleaved with
half-P@V-groups of earlier q-blocks; P@V of q-block 3 drains as the
tail. Pointwise: exp on ScalarE; subs split DVE/GpSimd; evictions DVE;
VAb->VA fp8 cast (DMA) + final scaled muls on GpSimd.
"""

import sys

sys.path.insert(0, "/opt/trn_rl_repo")

import ml_dtypes
import numpy as np

import concourse.bass as bass
import concourse.mybir as mybir
import concourse.tile as tile
from concourse import bacc
from concourse.bass_utils import run_bass_kernel_spmd

B = 4
N = 4096
M = 4096
C = 256  # INPUT_CH
R = 512  # REF_CH
SCALE = C ** (-0.5)
KSC = 16.0  # Wk host prescale
VSC = 8.0  # Wv host prescale
LAM = SCALE / KSC  # exp() scale on the raw score psum
NQ = 2048  # query rows per core

F32 = mybir.dt.float32
BF16 = mybir.dt.bfloat16
F8 = mybir.dt.float8e4
NP_BF16 = ml_dtypes.bfloat16
NP_F8 = ml_dtypes.float8_e4m3
DR = mybir.MatmulPerfMode.DoubleRow
Exp = mybir.ActivationFunctionType.Exp
Copy = mybir.ActivationFunctionType.Copy
ALU = mybir.AluOpType

QB = 512  # query block (free dim of score matmuls)
N_QB = NQ // QB  # 4
N_MC = M // 128  # 32 key chunks
N_CC = C // 128  # 2 chunks of the model dim
N_RC = R // 128  # 4 chunks of the ref dim
STRIPE = 512  # ref rows per processing stripe
N_STRIPES = M // STRIPE  # 8
VROW = 272  # VA chunk stride (C+2 used, padded to a 16B multiple)

_cached = None


def _build():
    nc = bacc.Bacc("TRN2", target_bir_lowering=False, debug=False)

    xT8_d = nc.dram_tensor("xT8", [C, NQ], F8, kind="ExternalInput")
    refT_d = nc.dram_tensor("refT", [R, M], BF16, kind="ExternalInput")
    wq_d = nc.dram_tensor("wq", [C, C], BF16, kind="ExternalInput")
    wk_d = nc.dram_tensor("wk16", [C, R], BF16, kind="ExternalInput")
    wv_d = nc.dram_tensor("wv8", [C, R], BF16, kind="ExternalInput")
    woT_d = nc.dram_tensor("woT", [C, C], BF16, kind="ExternalInput")
    out_d = nc.dram_tensor("out", [NQ, C], F32, kind="ExternalOutput")

    scratch_d = nc.dram_tensor("scratch", [128, 2], F32)

    with tile.TileContext(nc) as tc:
        with tc.tile_pool(name="const", bufs=1) as pc:
            kT8 = pc.tile([128, N_CC, M], F8)  # 16*k''^T  [c, m]
            VA = pc.tile([128, N_MC, VROW], F8)  # [8V' | 1 | 1 | pad]
            VAb = pc.tile([128, N_MC, C + 2], BF16)  # bf16 copy for colsum
            xT8 = pc.tile([128, N_CC, NQ], F8)
            gT = pc.tile([128, N_RC, C], BF16)  # 16*G^T = (16Wk)^T @ Wq
            wvoT = pc.tile([128, N_RC, C], BF16)  # (8 Wvo)^T
            caug_sb = pc.tile([1, C + 2], F32)
            caug_b = pc.tile([128, C + 2], F32)
            ones1 = pc.tile([128, 1], BF16)

            _psS_cm = tc.tile_pool(name="psS", bufs=3, space="PSUM")
            _psA_cm = tc.tile_pool(name="psA", bufs=2, space="PSUM")
            _pst_cm = tc.tile_pool(name="stage", bufs=2)
            _pstg_cm = tc.tile_pool(name="expstage", bufs=4)
            _ppt_cm = tc.tile_pool(name="ptpool", bufs=3)
            _pout_cm = tc.tile_pool(name="attn_out", bufs=4)
            psS = _psS_cm.__enter__()
            psA = _psA_cm.__enter__()
            pst = _pst_cm.__enter__()
            pstg = _pstg_cm.__enter__()
            ppt = _ppt_cm.__enter__()
            pout = _pout_cm.__enter__()

            nc.gpsimd.memset(VA[:, :, C : C + 2], 1.0)
            nc.gpsimd.memset(VAb[:, :, C : C + 2], 1.0)
            nc.gpsimd.memset(ones1[:], 1.0)

            # --- PE warm-up: trips the HAM clock gate to 2.4 GHz while the
            # input DMAs land.
            wu = pst.tile([128, QB], BF16, tag="wu", bufs=1)
            nc.vector.memset(wu[:], 0.0)
            ps_wu = psA.tile([128, QB], F32, tag="aux")
            for _ in range(13):
                nc.tensor.matmul(ps_wu[:], wu[:, 0:128], wu[:], start=True, stop=True)
            wu_out = pst.tile([128, 2], F32, tag="wu_out", bufs=1)
            nc.vector.tensor_copy(wu_out[:], ps_wu[:, 0:2])
            nc.sync.dma_start(scratch_d[:], wu_out[:])

            # ---------------- weight loads (pre-transposed on host) -------
            wq = pst.tile([128, N_CC, C], BF16, tag="wq", bufs=1)
            nc.scalar.dma_start(wq[:], wq_d[:].rearrange("(a p) o -> p a o", p=128))
            wk = pst.tile([128, N_CC, R], BF16, tag="wk", bufs=1)
            nc.scalar.dma_start(wk[:], wk_d[:].rearrange("(a p) r -> p a r", p=128))
            wv = pst.tile([128, N_CC, R], BF16, tag="wv", bufs=1)
            nc.scalar.dma_start(wv[:], wv_d[:].rearrange("(a p) r -> p a r", p=128))
            woT = pst.tile([128, N_CC, C], BF16, tag="woT", bufs=1)
            nc.scalar.dma_start(woT[:], woT_d[:].rearrange("(a p) o -> p a o", p=128))

            nc.scalar.dma_start(xT8[:], xT8_d[:].rearrange("(j p) n -> p j n", p=128))

            # weight folds: gT[r,c] = sum_co 16Wk[co,r] Wq[co,c];
            # WvoT[r,c'] = sum_c 8Wv[c,r] Wo[c',c]; two rj per aux tile.
            for dst, lhs, rhs_w in ((gT, wk, wq), (wvoT, wv, woT)):
                for pair in range(2):
                    ps = psA.tile([128, QB], F32, tag="aux", name="ps")
                    for half in range(2):
                        rj = 2 * pair + half
                        for a in range(N_CC):
                            nc.tensor.matmul(
                                ps[:, half * C : (half + 1) * C],
                                lhs[:, a, rj * 128 : (rj + 1) * 128],
                                rhs_w[:, a, :],
                                start=(a == 0),
                                stop=(a == N_CC - 1),
                            )
                    nc.scalar.activation(
                        dst[:, 2 * pair : 2 * pair + 2, :],
                        ps[:].rearrange("p (a c) -> p a c", a=2),
                        Copy,
                    )

            PT = {}
            refT_tiles = {}

            def dma_refT(s):
                if s >= N_STRIPES:
                    return
                t = pst.tile([128, N_RC, STRIPE], BF16, tag="refT", bufs=4)
                m0 = s * STRIPE
                nc.sync.dma_start(
                    t[:],
                    refT_d[:, m0 : m0 + STRIPE].rearrange("(j p) m -> p j m", p=128),
                )
                refT_tiles[s] = t

            def proj_piece(s, piece):
                # piece 0/1: kT chunk a; piece 2/3: V' chunk pair
                if s >= N_STRIPES:
                    return
                refT = refT_tiles[s]
                m0 = s * STRIPE
                ps = psA.tile([128, QB], F32, tag="aux", name="ps")
                if piece < 2:
                    a = piece
                    for j in range(N_RC):
                        nc.tensor.matmul(
                            ps[:],
                            gT[:, j, a * 128 : (a + 1) * 128],
                            refT[:, j, :],
                            start=(j == 0),
                            stop=(j == N_RC - 1),
                        )
                    nc.vector.tensor_copy(kT8[:, a, m0 : m0 + STRIPE], ps[:])
                else:
                    p2 = piece - 2
                    for half in range(2):
                        mi = 2 * p2 + half
                        for j in range(N_RC):
                            nc.tensor.matmul(
                                ps[:, half * C : (half + 1) * C],
                                refT[:, j, mi * 128 : (mi + 1) * 128],
                                wvoT[:, j, :],
                                start=(j == 0),
                                stop=(j == N_RC - 1),
                            )
                    mc0 = 4 * s + 2 * p2
                    nc.vector.tensor_copy(
                        VAb[:, mc0 : mc0 + 2, 0:C],
                        ps[:].rearrange("p (a c) -> p a c", a=2),
                    )
                    nc.gpsimd.dma_start(
                        VA[:, mc0 : mc0 + 2, 0:C], VAb[:, mc0 : mc0 + 2, 0:C]
                    )
                    if piece == 3:
                        refT_tiles.pop(s)

            def scores_tile(qb, t, on_gp=False):
                # scores for q-block qb vs key chunks 2t,2t+1 -> [128,1024]
                ps = psS.tile([128, 2 * QB], F32, tag="sps", name="ps")
                for h in range(2):
                    mc = 2 * t + h
                    nc.tensor.matmul(
                        ps[:, h * QB : (h + 1) * QB],
                        kT8[:, 0:2, mc * 128 : (mc + 1) * 128],
                        xT8[:, 0:2, qb * QB : (qb + 1) * QB],
                        start=True,
                        stop=True,
                        perf_mode=DR,
                    )
                stg = pstg.tile([128, 2 * QB], F32, tag="stg", bufs=4)
                nc.scalar.activation(stg[:], ps[:], Exp, scale=float(LAM))
                eng = nc.gpsimd if on_gp else nc.vector
                eng.tensor_scalar(
                    PT[qb][:, 2 * t : 2 * t + 2, :],
                    stg[:].rearrange("p (a q) -> p a q", a=2),
                    -1.0,
                    8.0,
                    op0=ALU.add,
                    op1=ALU.mult,
                )

            def pv_start(qb, qs):
                ps = psA.tile([128, QB], F32, tag="aux", name="pv")
                for i in range(8):
                    nc.tensor.matmul(
                        ps[:, 0 : C + 2],
                        PT[qb][:, 2 * i : 2 * i + 2, qs * 128 : (qs + 1) * 128],
                        VA[:, 2 * i : 2 * i + 2, 0 : C + 2],
                        start=(i == 0),
                        stop=False,
                        perf_mode=DR,
                    )
                return ps

            def pv_end(qb, qs, ps):
                for i in range(8, N_MC // 2):
                    nc.tensor.matmul(
                        ps[:, 0 : C + 2],
                        PT[qb][:, 2 * i : 2 * i + 2, qs * 128 : (qs + 1) * 128],
                        VA[:, 2 * i : 2 * i + 2, 0 : C + 2],
                        start=False,
                        stop=(i == N_MC // 2 - 1),
                        perf_mode=DR,
                    )
                numf = pout.tile([128, C + 2], F32, tag="numf", name="numf")
                nc.vector.scalar_tensor_tensor(
                    numf[:], ps[:, 0 : C + 2], 1.0, caug_b[:], op0=ALU.mult, op1=ALU.add
                )
                recip = pout.tile([128, 1], F32, tag="recip", name="recip")
                nc.vector.reciprocal(recip[:], numf[:, C : C + 1])
                o_sb = pout.tile([128, C], F32, tag="osb", name="o_sb")
                nc.gpsimd.tensor_scalar(
                    o_sb[:],
                    numf[:, 0:C],
                    recip[:],
                    1.0 / VSC,
                    op0=ALU.mult,
                    op1=ALU.mult,
                )
                r0 = qb * QB + qs * 128
                nc.sync.dma_start(out_d[r0 : r0 + 128, :], o_sb[:])

            # ---------------- phase 1: stripes, q-blocks 0-1 --------------
            PT[0] = ppt.tile([128, N_MC, QB], F8, tag="PT", name="PT0")
            PT[1] = ppt.tile([128, N_MC, QB], F8, tag="PT", name="PT1")
            dma_refT(0)
            dma_refT(1)
            dma_refT(2)
            for piece in range(4):
                proj_piece(0, piece)
            for piece in range(4):
                proj_piece(1, piece)
            for s in range(N_STRIPES):
                dma_refT(s + 3)
                scores_tile(0, 2 * s)
                proj_piece(s + 2, 0)
                scores_tile(0, 2 * s + 1)
                proj_piece(s + 2, 1)
                scores_tile(1, 2 * s, on_gp=True)
                proj_piece(s + 2, 2)
                scores_tile(1, 2 * s + 1, on_gp=True)
                proj_piece(s + 2, 3)
                if s == N_STRIPES - 1:
                    # colsum(V'') via ones^T @ VAb: caug = 8 * colsum
                    csps = psA.tile([128, QB], F32, tag="aux", name="cs")
                    for mc in range(N_MC):
                        nc.tensor.matmul(
                            csps[0:1, 0 : C + 2],
                            ones1[:],
                            VAb[:, mc, :],
                            start=(mc == 0),
                            stop=(mc == N_MC - 1),
                        )
                    nc.scalar.activation(
                        caug_sb[0:1, :], csps[0:1, 0 : C + 2], Copy, scale=float(VSC)
                    )
                    nc.gpsimd.partition_broadcast(caug_b[:], caug_sb[:])

            # ------- phase 2: scores qb2/qb3 pipelined with P@V 0-2 -------
            for qb in (2, 3):
                PT[qb] = ppt.tile([128, N_MC, QB], F8, tag="PT", name=f"PT{qb}")
                # P@V groups hosted in this qb-phase (from finished q-blocks)
                groups = (
                    [(0, 0), (0, 1), (0, 2), (0, 3), (1, 0), (1, 1), (1, 2), (1, 3)]
                    if qb == 2
                    else [(2, 0), (2, 1), (2, 2), (2, 3)]
                )
                pvs = {}
                for t in range(2 * N_STRIPES):
                    scores_tile(qb, t, on_gp=(t % 4 == 3))
                    gi = t // 2
                    if gi < len(groups):
                        if t % 2 == 0:
                            pvs[gi] = pv_start(*groups[gi])
                        else:
                            pv_end(*groups[gi], pvs.pop(gi))
            for qs in range(4):
                ps = pv_start(3, qs)
                pv_end(3, qs, ps)

            _pout_cm.__exit__(None, None, None)
            _ppt_cm.__exit__(None, None, None)
            _pstg_cm.__exit__(None, None, None)
            _pst_cm.__exit__(None, None, None)
            _psA_cm.__exit__(None, None, None)
            _psS_cm.__exit__(None, None, None)

    nc.compile()
    return nc


def _get_nc():
    global _cached
    if _cached is None:
        _cached = _build()
    return _cached


def kernel(x, ref, Wq, Wk, Wv, Wo, _trace=False, _trace_kwargs=None):
    nc = _get_nc()
    x = np.asarray(x, dtype=np.float32)
    ref = np.asarray(ref, dtype=np.float32)
    # host-side layout marshalling (transpose + cast + constant prescales on
    # the tiny weight tensors; no model FLOPs)
    wq_h = np.ascontiguousarray(np.asarray(Wq, np.float32).astype(NP_BF16))
    wk_h = np.ascontiguousarray((np.asarray(Wk, np.float32) * KSC).astype(NP_BF16))
    wv_h = np.ascontiguousarray((np.asarray(Wv, np.float32) * VSC).astype(NP_BF16))
    woT_h = np.ascontiguousarray(np.asarray(Wo, np.float32).T.astype(NP_BF16))
    refT_h = [np.ascontiguousarray(ref[b].T.astype(NP_BF16)) for b in range(B)]
    in_maps = []
    for core in range(8):
        b, h = divmod(core, 2)
        xT8_h = np.ascontiguousarray(x[b, h * NQ : (h + 1) * NQ, :].T.astype(NP_F8))
        in_maps.append(
            {
                "xT8": xT8_h,
                "refT": refT_h[b],
                "wq": wq_h,
                "wk16": wk_h,
                "wv8": wv_h,
                "woT": woT_h,
            }
        )
    res = run_bass_kernel_spmd(
        nc, in_maps, list(range(8)), trace=_trace, **(_trace_kwargs or {})
    )
    kernel.last_result = res
    out = np.empty((B, N, C), dtype=np.float32)
    for core in range(8):
        b, h = divmod(core, 2)
        out[b, h * NQ : (h + 1) * NQ, :] = res.results[core]["out"]
    return out


# revision 13
# speedup vs baseline: 1.2181x; 1.2130x over previous
"""Trainium2 Bass kernel for nn_BaseAttention (B=4, N=M=4096, C=256, R=512).

  q = x @ Wq.T;  k = ref @ Wk.T;  v = ref @ Wv.T
  out = softmax(q @ k.T / sqrt(C)) @ v @ Wo.T

Sharding: 8 cores; core i handles batch i//2, query rows (i%2)*2048..+2048.
K/V projection work is duplicated across the 2 cores of a batch (cheap).

v5: fp8(e4m3) DoubleRow matmuls for the two dominant phases (scores and
P@V run 2 fp8 MACs/cell/cycle). Precision is held by an expm1-style
decomposition: P' = 8*(exp(s)-1) is quantized to fp8 (error lands on the
small fluctuation term, not softmax's O(1) mean), P'@V'' accumulates in
fp8, and the exact rank-1 correction colsum(V_bf16) (ones^T @ VAb
matmuls) restores both the softmax mean term and the fp8-V quantization
loss. The k-projection stays bf16 (fp8 there pushes rel-err over the
gate); only the kT eviction quantizes to fp8. Scale factors (x16 on Wk,
x8 on Wv) keep fp8 operands in e4m3's normal range and are folded into
host weight prep, the exp scale, and the final output scale.

Schedule: PSUM = 3-deep ring of [128,1024] score tiles (6 banks) + a
2-deep [128,512] aux ring (2 banks) for projections / P@V / colsum, so
PE never blocks on an exp draining a ring slot. Phase 1 (8 stripes,
skew-2): scores for q-blocks 0-1 finely interleaved with stripe s+2's
projection pieces. Phase 2: scores for q-blocks 2-3 interleaved with
half-P@V-groups of earlier q-blocks; P@V of q-block 3 drains as the
tail. Pointwise: exp on ScalarE; subs split DVE/GpSimd; evictions DVE;
VAb->VA fp8 cast (DMA) + final scaled muls on GpSimd.
"""

import sys

sys.path.insert(0, "/opt/trn_rl_repo")

import ml_dtypes
import numpy as np

import concourse.bass as bass
import concourse.mybir as mybir
import concourse.tile as tile
from concourse import bacc
from concourse.bass_utils import run_bass_kernel_spmd

B = 4
N = 4096
M = 4096
C = 256  # INPUT_CH
R = 512  # REF_CH
SCALE = C ** (-0.5)
KSC = 16.0  # Wk host prescale
VSC = 8.0  # Wv host prescale
LAM = SCALE / KSC  # exp() scale on the raw score psum
NQ = 2048  # query rows per core

F32 = mybir.dt.float32
BF16 = mybir.dt.bfloat16
F8 = mybir.dt.float8e4
NP_BF16 = ml_dtypes.bfloat16
NP_F8 = ml_dtypes.float8_e4m3
DR = mybir.MatmulPerfMode.DoubleRow
Exp = mybir.ActivationFunctionType.Exp
Copy = mybir.ActivationFunctionType.Copy
ALU = mybir.AluOpType

QB = 512  # query block (free dim of score matmuls)
N_QB = NQ // QB  # 4
N_MC = M // 128  # 32 key chunks
N_CC = C // 128  # 2 chunks of the model dim
N_RC = R // 128  # 4 chunks of the ref dim
STRIPE = 512  # ref rows per processing stripe
N_STRIPES = M // STRIPE  # 8
VROW = 272  # VA chunk stride (C+2 used, padded to a 16B multiple)

_cached = None


def _build():
    nc = bacc.Bacc("TRN2", target_bir_lowering=False, debug=False)

    xT8_d = nc.dram_tensor("xT8", [C, NQ], F8, kind="ExternalInput")
    refT_d = nc.dram_tensor("refT", [R, M], BF16, kind="ExternalInput")
    wq_d = nc.dram_tensor("wq", [C, C], BF16, kind="ExternalInput")
    wk_d = nc.dram_tensor("wk16", [C, R], BF16, kind="ExternalInput")
    wv_d = nc.dram_tensor("wv8", [C, R], BF16, kind="ExternalInput")
    woT_d = nc.dram_tensor("woT", [C, C], BF16, kind="ExternalInput")
    out_d = nc.dram_tensor("out", [NQ, C], F32, kind="ExternalOutput")

    scratch_d = nc.dram_tensor("scratch", [128, 2], F32)

    with tile.TileContext(nc) as tc:
        with tc.tile_pool(name="const", bufs=1) as pc:
            kT8 = pc.tile([128, N_CC, M], F8)  # 16*k''^T  [c, m]
            VA = pc.tile([128, N_MC, VROW], F8)  # [8V' | 1 | 1 | pad]
            VAb = pc.tile([128, N_MC, C + 2], BF16)  # bf16 copy for colsum
            xT8 = pc.tile([128, N_CC, NQ], F8)
            gT = pc.tile([128, N_RC, C], BF16)  # 16*G^T = (16Wk)^T @ Wq
            wvoT = pc.tile([128, N_RC, C], BF16)  # (8 Wvo)^T
            caug_sb = pc.tile([1, C + 2], F32)
            caug_b = pc.tile([128, C + 2], F32)
            ones1 = pc.tile([128, 1], BF16)

            _psS_cm = tc.tile_pool(name="psS", bufs=3, space="PSUM")
            _psA_cm = tc.tile_pool(name="psA", bufs=2, space="PSUM")
            _pst_cm = tc.tile_pool(name="stage", bufs=2)
            _pstg_cm = tc.tile_pool(name="expstage", bufs=4)
            _ppt_cm = tc.tile_pool(name="ptpool", bufs=3)
            _pout_cm = tc.tile_pool(name="attn_out", bufs=4)
            psS = _psS_cm.__enter__()
            psA = _psA_cm.__enter__()
            pst = _pst_cm.__enter__()
            pstg = _pstg_cm.__enter__()
            ppt = _ppt_cm.__enter__()
            pout = _pout_cm.__enter__()

            nc.gpsimd.memset(VA[:, :, C : C + 2], 1.0)
            nc.gpsimd.memset(VAb[:, :, C : C + 2], 1.0)
            nc.gpsimd.memset(ones1[:], 1.0)

            # --- PE warm-up: trips the HAM clock gate to 2.4 GHz while the
            # input DMAs land.
            wu = pst.tile([128, QB], BF16, tag="wu", bufs=1)
            nc.vector.memset(wu[:], 0.0)
            ps_wu = psA.tile([128, QB], F32, tag="aux")
            for _ in range(13):
                nc.tensor.matmul(ps_wu[:], wu[:, 0:128], wu[:], start=True, stop=True)
            wu_out = pst.tile([128, 2], F32, tag="wu_out", bufs=1)
            nc.vector.tensor_copy(wu_out[:], ps_wu[:, 0:2])
            nc.sync.dma_start(scratch_d[:], wu_out[:])

            # ---------------- weight loads (pre-transposed on host) -------
            wq = pst.tile([128, N_CC, C], BF16, tag="wq", bufs=1)
            nc.scalar.dma_start(wq[:], wq_d[:].rearrange("(a p) o -> p a o", p=128))
            wk = pst.tile([128, N_CC, R], BF16, tag="wk", bufs=1)
            nc.scalar.dma_start(wk[:], wk_d[:].rearrange("(a p) r -> p a r", p=128))
            wv = pst.tile([128, N_CC, R], BF16, tag="wv", bufs=1)
            nc.scalar.dma_start(wv[:], wv_d[:].rearrange("(a p) r -> p a r", p=128))
            woT = pst.tile([128, N_CC, C], BF16, tag="woT", bufs=1)
            nc.scalar.dma_start(woT[:], woT_d[:].rearrange("(a p) o -> p a o", p=128))

            nc.scalar.dma_start(xT8[:], xT8_d[:].rearrange("(j p) n -> p j n", p=128))

            # weight folds: gT[r,c] = sum_co 16Wk[co,r] Wq[co,c];
            # WvoT[r,c'] = sum_c 8Wv[c,r] Wo[c',c]; two rj per aux tile.
            for dst, lhs, rhs_w in ((gT, wk, wq), (wvoT, wv, woT)):
                for pair in range(2):
                    ps = psA.tile([128, QB], F32, tag="aux", name="ps")
                    for half in range(2):
                        rj = 2 * pair + half
                        for a in range(N_CC):
                            nc.tensor.matmul(
                                ps[:, half * C : (half + 1) * C],
                                lhs[:, a, rj * 128 : (rj + 1) * 128],
                                rhs_w[:, a, :],
                                start=(a == 0),
                                stop=(a == N_CC - 1),
                            )
                    nc.scalar.activation(
                        dst[:, 2 * pair : 2 * pair + 2, :],
                        ps[:].rearrange("p (a c) -> p a c", a=2),
                        Copy,
                    )

            PT = {}
            refT_tiles = {}

            def dma_refT(s):
                if s >= N_STRIPES:
                    return
                t = pst.tile([128, N_RC, STRIPE], BF16, tag="refT", bufs=4)
                m0 = s * STRIPE
                nc.sync.dma_start(
                    t[:],
                    refT_d[:, m0 : m0 + STRIPE].rearrange("(j p) m -> p j m", p=128),
                )
                refT_tiles[s] = t

            def proj_piece(s, piece):
                # piece 0/1: kT chunk a; piece 2/3: V' chunk pair
                if s >= N_STRIPES:
                    return
                refT = refT_tiles[s]
                m0 = s * STRIPE
                ps = psA.tile([128, QB], F32, tag="aux", name="ps")
                if piece < 2:
                    a = piece
                    for j in range(N_RC):
                        nc.tensor.matmul(
                            ps[:],
                            gT[:, j, a * 128 : (a + 1) * 128],
                            refT[:, j, :],
                            start=(j == 0),
                            stop=(j == N_RC - 1),
                        )
                    nc.vector.tensor_copy(kT8[:, a, m0 : m0 + STRIPE], ps[:])
                else:
                    p2 = piece - 2
                    for half in range(2):
                        mi = 2 * p2 + half
                        for j in range(N_RC):
                            nc.tensor.matmul(
                                ps[:, half * C : (half + 1) * C],
                                refT[:, j, mi * 128 : (mi + 1) * 128],
                                wvoT[:, j, :],
                                start=(j == 0),
                                stop=(j == N_RC - 1),
                            )
                    mc0 = 4 * s + 2 * p2
                    nc.vector.tensor_copy(
                        VAb[:, mc0 : mc0 + 2, 0:C],
                        ps[:].rearrange("p (a c) -> p a c", a=2),
                    )
                    nc.gpsimd.dma_start(
                        VA[:, mc0 : mc0 + 2, 0:C], VAb[:, mc0 : mc0 + 2, 0:C]
                    )
                    if piece == 3:
                        refT_tiles.pop(s)

            def scores_tile(qb, t, on_gp=False):
                # scores for q-block qb vs key chunks 2t,2t+1 -> [128,1024]
                ps = psS.tile([128, 2 * QB], F32, tag="sps", name="ps")
                for h in range(2):
                    mc = 2 * t + h
                    nc.tensor.matmul(
                        ps[:, h * QB : (h + 1) * QB],
                        kT8[:, 0:2, mc * 128 : (mc + 1) * 128],
                        xT8[:, 0:2, qb * QB : (qb + 1) * QB],
                        start=True,
                        stop=True,
                        perf_mode=DR,
                    )
                stg = pstg.tile([128, 2 * QB], F32, tag="stg", bufs=4)
                nc.scalar.activation(stg[:], ps[:], Exp, scale=float(LAM))
                eng = nc.gpsimd if on_gp else nc.vector
                eng.tensor_scalar(
                    PT[qb][:, 2 * t : 2 * t + 2, :],
                    stg[:].rearrange("p (a q) -> p a q", a=2),
                    -1.0,
                    8.0,
                    op0=ALU.add,
                    op1=ALU.mult,
                )

            def pv_start(qb, qs):
                ps = psA.tile([128, QB], F32, tag="aux", name="pv")
                for i in range(8):
                    nc.tensor.matmul(
                        ps[:, 0 : C + 2],
                        PT[qb][:, 2 * i : 2 * i + 2, qs * 128 : (qs + 1) * 128],
                        VA[:, 2 * i : 2 * i + 2, 0 : C + 2],
                        start=(i == 0),
                        stop=False,
                        perf_mode=DR,
                    )
                return ps

            def pv_end(qb, qs, ps):
                for i in range(8, N_MC // 2):
                    nc.tensor.matmul(
                        ps[:, 0 : C + 2],
                        PT[qb][:, 2 * i : 2 * i + 2, qs * 128 : (qs + 1) * 128],
                        VA[:, 2 * i : 2 * i + 2, 0 : C + 2],
                        start=False,
                        stop=(i == N_MC // 2 - 1),
                        perf_mode=DR,
                    )
                numf = pout.tile([128, C + 2], F32, tag="numf", name="numf")
                nc.vector.scalar_tensor_tensor(
                    numf[:], ps[:, 0 : C + 2], 1.0, caug_b[:], op0=ALU.mult, op1=ALU.add
                )
                recip = pout.tile([128, 1], F32, tag="recip", name="recip")
                nc.vector.reciprocal(recip[:], numf[:, C : C + 1])
                o_sb = pout.tile([128, C], F32, tag="osb", name="o_sb")
                nc.gpsimd.tensor_scalar(
                    o_sb[:],
                    numf[:, 0:C],
                    recip[:],
                    1.0 / VSC,
                    op0=ALU.mult,
                    op1=ALU.mult,
                )
                r0 = qb * QB + qs * 128
                nc.sync.dma_start(out_d[r0 : r0 + 128, :], o_sb[:])

            # ---------------- phase 1: stripes, q-blocks 0-1 --------------
            PT[0] = ppt.tile([128, N_MC, QB], F8, tag="PT", name="PT0")
            PT[1] = ppt.tile([128, N_MC, QB], F8, tag="PT", name="PT1")
            dma_refT(0)
            dma_refT(1)
            dma_refT(2)
            for piece in range(4):
                proj_piece(0, piece)
            for piece in range(4):
                proj_piece(1, piece)
            for s in range(N_STRIPES):
                dma_refT(s + 3)
                scores_tile(0, 2 * s)
                proj_piece(s + 2, 0)
                scores_tile(0, 2 * s + 1)
                proj_piece(s + 2, 1)
                scores_tile(1, 2 * s, on_gp=True)
                proj_piece(s + 2, 2)
                scores_tile(1, 2 * s + 1, on_gp=True)
                proj_piece(s + 2, 3)
                if s == N_STRIPES - 2:
                    # colsum(V'') via ones^T @ VAb: caug = 8 * colsum
                    csps = psA.tile([128, QB], F32, tag="aux", name="cs")
                    for mc in range(N_MC):
                        nc.tensor.matmul(
                            csps[0:1, 0 : C + 2],
                            ones1[:],
                            VAb[:, mc, :],
                            start=(mc == 0),
                            stop=(mc == N_MC - 1),
                        )
                    nc.scalar.activation(
                        caug_sb[0:1, :], csps[0:1, 0 : C + 2], Copy, scale=float(VSC)
                    )
                    nc.gpsimd.partition_broadcast(caug_b[:], caug_sb[:])

            # ------- phase 2: scores qb2/qb3 pipelined with P@V 0-2 -------
            pvs3 = {}
            for qb in (2, 3):
                PT[qb] = ppt.tile([128, N_MC, QB], F8, tag="PT", name=f"PT{qb}")
                # P@V groups hosted in this qb-phase (from finished q-blocks)
                groups = (
                    [(0, 0), (0, 1), (0, 2), (0, 3), (1, 0), (1, 1), (1, 2), (1, 3)]
                    if qb == 2
                    else [(2, 0), (2, 1), (2, 2), (2, 3)]
                )
                for t in range(2 * N_STRIPES):
                    scores_tile(qb, t, on_gp=(t % 4 == 3))
                    if t % 2 == 1:
                        gi = t // 2
                        if gi < len(groups):
                            ps = pv_start(*groups[gi])
                            pv_end(*groups[gi], ps)
                        elif qb == 3 and gi - 4 < 2:
                            # first half of P@V(qb3) for 2 groups (aux ring
                            # depth): key chunks 0..15 are ready once scores
                            # tiles 0..7 are written
                            qs = gi - 4
                            pvs3[qs] = pv_start(3, qs)
            for qs in range(4):
                if qs in pvs3:
                    pv_end(3, qs, pvs3.pop(qs))
                else:
                    ps = pv_start(3, qs)
                    pv_end(3, qs, ps)

            _pout_cm.__exit__(None, None, None)
            _ppt_cm.__exit__(None, None, None)
            _pstg_cm.__exit__(None, None, None)
            _pst_cm.__exit__(None, None, None)
            _psA_cm.__exit__(None, None, None)
            _psS_cm.__exit__(None, None, None)

    nc.compile()
    return nc


def _get_nc():
    global _cached
    if _cached is None:
        _cached = _build()
    return _cached


def kernel(x, ref, Wq, Wk, Wv, Wo, _trace=False, _trace_kwargs=None):
    nc = _get_nc()
    x = np.asarray(x, dtype=np.float32)
    ref = np.asarray(ref, dtype=np.float32)
    # host-side layout marshalling (transpose + cast + constant prescales on
    # the tiny weight tensors; no model FLOPs)
    wq_h = np.ascontiguousarray(np.asarray(Wq, np.float32).astype(NP_BF16))
    wk_h = np.ascontiguousarray((np.asarray(Wk, np.float32) * KSC).astype(NP_BF16))
    wv_h = np.ascontiguousarray((np.asarray(Wv, np.float32) * VSC).astype(NP_BF16))
    woT_h = np.ascontiguousarray(np.asarray(Wo, np.float32).T.astype(NP_BF16))
    refT_h = [np.ascontiguousarray(ref[b].T.astype(NP_BF16)) for b in range(B)]
    in_maps = []
    for core in range(8):
        b, h = divmod(core, 2)
        xT8_h = np.ascontiguousarray(x[b, h * NQ : (h + 1) * NQ, :].T.astype(NP_F8))
        in_maps.append(
            {
                "xT8": xT8_h,
                "refT": refT_h[b],
                "wq": wq_h,
                "wk16": wk_h,
                "wv8": wv_h,
                "woT": woT_h,
            }
        )
    res = run_bass_kernel_spmd(
        nc, in_maps, list(range(8)), trace=_trace, **(_trace_kwargs or {})
    )
    kernel.last_result = res
    out = np.empty((B, N, C), dtype=np.float32)
    for core in range(8):
        b, h = divmod(core, 2)
        out[b, h * NQ : (h + 1) * NQ, :] = res.results[core]["out"]
    return out


# revision 15
# speedup vs baseline: 1.2266x; 1.0070x over previous
"""Trainium2 Bass kernel for nn_BaseAttention (B=4, N=M=4096, C=256, R=512).

  q = x @ Wq.T;  k = ref @ Wk.T;  v = ref @ Wv.T
  out = softmax(q @ k.T / sqrt(C)) @ v @ Wo.T

Sharding: 8 cores; core i handles batch i//2, query rows (i%2)*2048..+2048.
K/V projection work is duplicated across the 2 cores of a batch (cheap).

v5: fp8(e4m3) DoubleRow matmuls for the two dominant phases (scores and
P@V run 2 fp8 MACs/cell/cycle). Precision is held by an expm1-style
decomposition: P' = 8*(exp(s)-1) is quantized to fp8 (error lands on the
small fluctuation term, not softmax's O(1) mean), P'@V'' accumulates in
fp8, and the exact rank-1 correction colsum(V_bf16) (ones^T @ VAb
matmuls) restores both the softmax mean term and the fp8-V quantization
loss. The k-projection stays bf16 (fp8 there pushes rel-err over the
gate); only the kT eviction quantizes to fp8. Scale factors (x16 on Wk,
x8 on Wv) keep fp8 operands in e4m3's normal range and are folded into
host weight prep, the exp scale, and the final output scale.

Schedule: PSUM = 3-deep ring of [128,1024] score tiles (6 banks) + a
2-deep [128,512] aux ring (2 banks) for projections / P@V / colsum, so
PE never blocks on an exp draining a ring slot. Phase 1 (8 stripes,
skew-2): scores for q-blocks 0-1 finely interleaved with stripe s+2's
projection pieces. Phase 2: scores for q-blocks 2-3 interleaved with
half-P@V-groups of earlier q-blocks; P@V of q-block 3 drains as the
tail. Pointwise: exp on ScalarE; subs split DVE/GpSimd; evictions DVE;
VAb->VA fp8 cast (DMA) + final scaled muls on GpSimd.
"""

import sys

sys.path.insert(0, "/opt/trn_rl_repo")

import ml_dtypes
import numpy as np

import concourse.bass as bass
import concourse.mybir as mybir
import concourse.tile as tile
from concourse import bacc
from concourse.bass_utils import run_bass_kernel_spmd

B = 4
N = 4096
M = 4096
C = 256  # INPUT_CH
R = 512  # REF_CH
SCALE = C ** (-0.5)
KSC = 16.0  # Wk host prescale
VSC = 8.0  # Wv host prescale
LAM = SCALE / KSC  # exp() scale on the raw score psum
NQ = 2048  # query rows per core

F32 = mybir.dt.float32
BF16 = mybir.dt.bfloat16
F8 = mybir.dt.float8e4
NP_BF16 = ml_dtypes.bfloat16
NP_F8 = ml_dtypes.float8_e4m3
DR = mybir.MatmulPerfMode.DoubleRow
Exp = mybir.ActivationFunctionType.Exp
Copy = mybir.ActivationFunctionType.Copy
ALU = mybir.AluOpType

QB = 512  # query block (free dim of score matmuls)
N_QB = NQ // QB  # 4
N_MC = M // 128  # 32 key chunks
N_CC = C // 128  # 2 chunks of the model dim
N_RC = R // 128  # 4 chunks of the ref dim
STRIPE = 512  # ref rows per processing stripe
N_STRIPES = M // STRIPE  # 8
VROW = 272  # VA chunk stride (C+2 used, padded to a 16B multiple)

_cached = None


def _build():
    nc = bacc.Bacc("TRN2", target_bir_lowering=False, debug=False)

    xT8_d = nc.dram_tensor("xT8", [C, NQ], F8, kind="ExternalInput")
    refT_d = nc.dram_tensor("refT", [R, M], BF16, kind="ExternalInput")
    wq_d = nc.dram_tensor("wq", [C, C], BF16, kind="ExternalInput")
    wk_d = nc.dram_tensor("wk16", [C, R], BF16, kind="ExternalInput")
    wv_d = nc.dram_tensor("wv8", [C, R], BF16, kind="ExternalInput")
    woT_d = nc.dram_tensor("woT", [C, C], BF16, kind="ExternalInput")
    out_d = nc.dram_tensor("out", [NQ, C], F32, kind="ExternalOutput")

    scratch_d = nc.dram_tensor("scratch", [128, 2], F32)

    with tile.TileContext(nc) as tc:
        with tc.tile_pool(name="const", bufs=1) as pc:
            kT8 = pc.tile([128, N_CC, M], F8)  # 16*k''^T  [c, m]
            VA = pc.tile([128, N_MC, VROW], F8)  # [8V' | 1 | 1 | pad]
            VAb = pc.tile([128, N_MC, C + 2], BF16)  # bf16 copy for colsum
            xT8 = pc.tile([128, N_CC, NQ], F8)
            gT = pc.tile([128, N_RC, C], BF16)  # 16*G^T = (16Wk)^T @ Wq
            wvoT = pc.tile([128, N_RC, C], BF16)  # (8 Wvo)^T
            caug_sb = pc.tile([1, C + 2], F32)
            caug_b = pc.tile([128, C + 2], F32)
            ones1 = pc.tile([128, 1], BF16)

            _psS_cm = tc.tile_pool(name="psS", bufs=3, space="PSUM")
            _psA_cm = tc.tile_pool(name="psA", bufs=2, space="PSUM")
            _pst_cm = tc.tile_pool(name="stage", bufs=2)
            _pstg_cm = tc.tile_pool(name="expstage", bufs=4)
            _ppt_cm = tc.tile_pool(name="ptpool", bufs=3)
            _pout_cm = tc.tile_pool(name="attn_out", bufs=4)
            psS = _psS_cm.__enter__()
            psA = _psA_cm.__enter__()
            pst = _pst_cm.__enter__()
            pstg = _pstg_cm.__enter__()
            ppt = _ppt_cm.__enter__()
            pout = _pout_cm.__enter__()

            nc.gpsimd.memset(VA[:, :, C : C + 2], 1.0)
            nc.gpsimd.memset(VAb[:, :, C : C + 2], 1.0)
            nc.gpsimd.memset(ones1[:], 1.0)

            # --- PE warm-up: trips the HAM clock gate to 2.4 GHz while the
            # input DMAs land.
            wu = pst.tile([128, QB], BF16, tag="wu", bufs=1)
            nc.vector.memset(wu[:], 0.0)
            ps_wu = psA.tile([128, QB], F32, tag="aux")
            for _ in range(13):
                nc.tensor.matmul(ps_wu[:], wu[:, 0:128], wu[:], start=True, stop=True)
            wu_out = pst.tile([128, 2], F32, tag="wu_out", bufs=1)
            nc.vector.tensor_copy(wu_out[:], ps_wu[:, 0:2])
            nc.sync.dma_start(scratch_d[:], wu_out[:])

            # ---------------- weight loads (pre-transposed on host) -------
            wq = pst.tile([128, N_CC, C], BF16, tag="wq", bufs=1)
            nc.scalar.dma_start(wq[:], wq_d[:].rearrange("(a p) o -> p a o", p=128))
            wk = pst.tile([128, N_CC, R], BF16, tag="wk", bufs=1)
            nc.scalar.dma_start(wk[:], wk_d[:].rearrange("(a p) r -> p a r", p=128))
            wv = pst.tile([128, N_CC, R], BF16, tag="wv", bufs=1)
            nc.scalar.dma_start(wv[:], wv_d[:].rearrange("(a p) r -> p a r", p=128))
            woT = pst.tile([128, N_CC, C], BF16, tag="woT", bufs=1)
            nc.scalar.dma_start(woT[:], woT_d[:].rearrange("(a p) o -> p a o", p=128))

            nc.scalar.dma_start(xT8[:], xT8_d[:].rearrange("(j p) n -> p j n", p=128))

            # weight folds: gT[r,c] = sum_co 16Wk[co,r] Wq[co,c];
            # WvoT[r,c'] = sum_c 8Wv[c,r] Wo[c',c]; two rj per aux tile.
            for dst, lhs, rhs_w in ((gT, wk, wq), (wvoT, wv, woT)):
                for pair in range(2):
                    ps = psA.tile([128, QB], F32, tag="aux", name="ps")
                    for half in range(2):
                        rj = 2 * pair + half
                        for a in range(N_CC):
                            nc.tensor.matmul(
                                ps[:, half * C : (half + 1) * C],
                                lhs[:, a, rj * 128 : (rj + 1) * 128],
                                rhs_w[:, a, :],
                                start=(a == 0),
                                stop=(a == N_CC - 1),
                            )
                    nc.scalar.activation(
                        dst[:, 2 * pair : 2 * pair + 2, :],
                        ps[:].rearrange("p (a c) -> p a c", a=2),
                        Copy,
                    )

            PT = {}
            refT_tiles = {}

            def dma_refT(s):
                if s >= N_STRIPES:
                    return
                t = pst.tile([128, N_RC, STRIPE], BF16, tag="refT", bufs=4)
                m0 = s * STRIPE
                nc.sync.dma_start(
                    t[:],
                    refT_d[:, m0 : m0 + STRIPE].rearrange("(j p) m -> p j m", p=128),
                )
                refT_tiles[s] = t

            def proj_piece(s, piece):
                # piece 0/1: kT chunk a; piece 2/3: V' chunk pair
                if s >= N_STRIPES:
                    return
                refT = refT_tiles[s]
                m0 = s * STRIPE
                ps = psA.tile([128, QB], F32, tag="aux", name="ps")
                if piece < 2:
                    a = piece
                    for j in range(N_RC):
                        nc.tensor.matmul(
                            ps[:],
                            gT[:, j, a * 128 : (a + 1) * 128],
                            refT[:, j, :],
                            start=(j == 0),
                            stop=(j == N_RC - 1),
                        )
                    nc.vector.tensor_copy(kT8[:, a, m0 : m0 + STRIPE], ps[:])
                else:
                    p2 = piece - 2
                    for half in range(2):
                        mi = 2 * p2 + half
                        for j in range(N_RC):
                            nc.tensor.matmul(
                                ps[:, half * C : (half + 1) * C],
                                refT[:, j, mi * 128 : (mi + 1) * 128],
                                wvoT[:, j, :],
                                start=(j == 0),
                                stop=(j == N_RC - 1),
                            )
                    mc0 = 4 * s + 2 * p2
                    nc.vector.tensor_copy(
                        VAb[:, mc0 : mc0 + 2, 0:C],
                        ps[:].rearrange("p (a c) -> p a c", a=2),
                    )
                    nc.gpsimd.dma_start(
                        VA[:, mc0 : mc0 + 2, 0:C], VAb[:, mc0 : mc0 + 2, 0:C]
                    )
                    if piece == 3:
                        refT_tiles.pop(s)

            def scores_tile(qb, t, on_gp=False):
                # scores for q-block qb vs key chunks 2t,2t+1 -> [128,1024]
                ps = psS.tile([128, 2 * QB], F32, tag="sps", name="ps")
                for h in range(2):
                    mc = 2 * t + h
                    nc.tensor.matmul(
                        ps[:, h * QB : (h + 1) * QB],
                        kT8[:, 0:2, mc * 128 : (mc + 1) * 128],
                        xT8[:, 0:2, qb * QB : (qb + 1) * QB],
                        start=True,
                        stop=True,
                        perf_mode=DR,
                    )
                stg = pstg.tile([128, 2 * QB], F32, tag="stg", bufs=4)
                nc.scalar.activation(stg[:], ps[:], Exp, scale=float(LAM))
                eng = nc.gpsimd if on_gp else nc.vector
                eng.tensor_scalar(
                    PT[qb][:, 2 * t : 2 * t + 2, :],
                    stg[:].rearrange("p (a q) -> p a q", a=2),
                    -1.0,
                    8.0,
                    op0=ALU.add,
                    op1=ALU.mult,
                )

            def pv_start(qb, qs):
                ps = psA.tile([128, QB], F32, tag="aux", name="pv")
                for i in range(8):
                    nc.tensor.matmul(
                        ps[:, 0 : C + 2],
                        PT[qb][:, 2 * i : 2 * i + 2, qs * 128 : (qs + 1) * 128],
                        VA[:, 2 * i : 2 * i + 2, 0 : C + 2],
                        start=(i == 0),
                        stop=False,
                        perf_mode=DR,
                    )
                return ps

            def pv_end(qb, qs, ps):
                for i in range(8, N_MC // 2):
                    nc.tensor.matmul(
                        ps[:, 0 : C + 2],
                        PT[qb][:, 2 * i : 2 * i + 2, qs * 128 : (qs + 1) * 128],
                        VA[:, 2 * i : 2 * i + 2, 0 : C + 2],
                        start=False,
                        stop=(i == N_MC // 2 - 1),
                        perf_mode=DR,
                    )
                numf = pout.tile([128, C + 2], F32, tag="numf", name="numf")
                nc.vector.scalar_tensor_tensor(
                    numf[:], ps[:, 0 : C + 2], 1.0, caug_b[:], op0=ALU.mult, op1=ALU.add
                )
                recip = pout.tile([128, 1], F32, tag="recip", name="recip")
                nc.vector.reciprocal(recip[:], numf[:, C : C + 1])
                o_sb = pout.tile([128, C], F32, tag="osb", name="o_sb")
                nc.gpsimd.tensor_scalar(
                    o_sb[:],
                    numf[:, 0:C],
                    recip[:],
                    1.0 / VSC,
                    op0=ALU.mult,
                    op1=ALU.mult,
                )
                r0 = qb * QB + qs * 128
                nc.sync.dma_start(out_d[r0 : r0 + 128, :], o_sb[:])

            # ---------------- phase 1: stripes, q-blocks 0-1 --------------
            PT[0] = ppt.tile([128, N_MC, QB], F8, tag="PT", name="PT0")
            PT[1] = ppt.tile([128, N_MC, QB], F8, tag="PT", name="PT1")
            dma_refT(0)
            dma_refT(1)
            dma_refT(2)
            for piece in range(4):
                proj_piece(0, piece)
            for piece in range(4):
                proj_piece(1, piece)
            PT[2] = ppt.tile([128, N_MC, QB], F8, tag="PT", name="PT2")
            for s in range(N_STRIPES):
                dma_refT(s + 3)
                scores_tile(0, 2 * s)
                proj_piece(s + 2, 0)
                if s == N_STRIPES - 1:
                    scores_tile(2, 0)
                scores_tile(0, 2 * s + 1)
                proj_piece(s + 2, 1)
                if s == N_STRIPES - 1:
                    scores_tile(2, 1)
                scores_tile(1, 2 * s, on_gp=True)
                proj_piece(s + 2, 2)
                if s == N_STRIPES - 1:
                    scores_tile(2, 2)
                scores_tile(1, 2 * s + 1, on_gp=True)
                proj_piece(s + 2, 3)
                if s == N_STRIPES - 1:
                    scores_tile(2, 3, on_gp=True)
                if s == N_STRIPES - 2:
                    # colsum(V'') via ones^T @ VAb: caug = 8 * colsum
                    csps = psA.tile([128, QB], F32, tag="aux", name="cs")
                    for mc in range(N_MC):
                        nc.tensor.matmul(
                            csps[0:1, 0 : C + 2],
                            ones1[:],
                            VAb[:, mc, :],
                            start=(mc == 0),
                            stop=(mc == N_MC - 1),
                        )
                    nc.scalar.activation(
                        caug_sb[0:1, :], csps[0:1, 0 : C + 2], Copy, scale=float(VSC)
                    )
                    nc.gpsimd.partition_broadcast(caug_b[:], caug_sb[:])

            # ------- phase 2: scores qb2/qb3 pipelined with P@V 0-2 -------
            pvs3 = {}
            for qb in (2, 3):
                if qb == 3:
                    PT[qb] = ppt.tile([128, N_MC, QB], F8, tag="PT", name=f"PT{qb}")
                # P@V groups hosted in this qb-phase (from finished q-blocks)
                groups = (
                    [(0, 0), (0, 1), (0, 2), (0, 3), (1, 0), (1, 1)]
                    if qb == 2
                    else [(1, 2), (1, 3), (2, 0), (2, 1), (2, 2), (2, 3)]
                )
                t0 = 4 if qb == 2 else 0
                for t in range(t0, 2 * N_STRIPES):
                    scores_tile(qb, t, on_gp=(t % 4 == 3))
                    if t % 2 == 1:
                        gi = (t - t0) // 2
                        if gi < len(groups):
                            ps = pv_start(*groups[gi])
                            pv_end(*groups[gi], ps)
                        elif qb == 3 and gi - len(groups) < 2:
                            # first half of P@V(qb3) for 2 groups (aux ring
                            # depth): key chunks 0..15 are ready once scores
                            # tiles 0..7 are written
                            qs = gi - len(groups)
                            pvs3[qs] = pv_start(3, qs)
            for qs in range(4):
                if qs in pvs3:
                    pv_end(3, qs, pvs3.pop(qs))
                else:
                    ps = pv_start(3, qs)
                    pv_end(3, qs, ps)

            _pout_cm.__exit__(None, None, None)
            _ppt_cm.__exit__(None, None, None)
            _pstg_cm.__exit__(None, None, None)
            _pst_cm.__exit__(None, None, None)
            _psA_cm.__exit__(None, None, None)
            _psS_cm.__exit__(None, None, None)

    nc.compile()
    return nc


def _get_nc():
    global _cached
    if _cached is None:
        _cached = _build()
    return _cached


def kernel(x, ref, Wq, Wk, Wv, Wo, _trace=False, _trace_kwargs=None):
    nc = _get_nc()
    x = np.asarray(x, dtype=np.float32)
    ref = np.asarray(ref, dtype=np.float32)
    # host-side layout marshalling (transpose + cast + constant prescales on
    # the tiny weight tensors; no model FLOPs)
    wq_h = np.ascontiguousarray(np.asarray(Wq, np.float32).astype(NP_BF16))
    wk_h = np.ascontiguousarray((np.asarray(Wk, np.float32) * KSC).astype(NP_BF16))
    wv_h = np.ascontiguousarray((np.asarray(Wv, np.float32) * VSC).astype(NP_BF16))
    woT_h = np.ascontiguousarray(np.asarray(Wo, np.float32).T.astype(NP_BF16))
    refT_h = [np.ascontiguousarray(ref[b].T.astype(NP_BF16)) for b in range(B)]
    in_maps = []
    for core in range(8):
        b, h = divmod(core, 2)
        xT8_h = np.ascontiguousarray(x[b, h * NQ : (h + 1) * NQ, :].T.astype(NP_F8))
        in_maps.append(
            {
                "xT8": xT8_h,
                "refT": refT_h[b],
                "wq": wq_h,
                "wk16": wk_h,
                "wv8": wv_h,
                "woT": woT_h,
            }
        )
    res = run_bass_kernel_spmd(
        nc, in_maps, list(range(8)), trace=_trace, **(_trace_kwargs or {})
    )
    kernel.last_result = res
    out = np.empty((B, N, C), dtype=np.float32)
    for core in range(8):
        b, h = divmod(core, 2)
        out[b, h * NQ : (h + 1) * NQ, :] = res.results[core]["out"]
    return out
